# revision 1
# baseline (speedup 1.0000x reference)
"""Boolean reservoir computing on 8 Trainium2 NeuronCores (Bass/Tile).

Algorithm per step (data-parallel over samples, 64 per core):
  - exact integer indices idx = r @ Wp.T via bf16-split matmuls accumulated
    in fp32 PSUM (input-bit injection folded into separate stationary
    weights, so no state scatter is needed)
  - LUT lookup from an SBUF-resident bitpacked table (16 payload bits per
    uint32 word, one node's table per partition) via the GPSIMD ap_gather
    instruction; per-lookup bit extracted with integer ALU ops and a
    power-of-two AND mask built from exponent-field arithmetic
  - final readout matmul in fp32 on-device
"""
import numpy as np

R = 256; LUT_LEN = 18; M = 512; S = 512; OUT = 10
KB = 32
NW = 1 << (LUT_LEN - 4)     # 16384 16-bit words per node
NE = 2 * NW                 # uint32 entries per partition (2 nodes)
N_CORES = 8
PER_CORE = M // N_CORES     # 64


def _host_prep(x, input_nodes, lut, W_res, primes, init_res, readout_W, readout_b):
    import ml_dtypes
    bf = ml_dtypes.bfloat16
    x = np.asarray(x); input_nodes = np.asarray(input_nodes)
    lut = np.asarray(lut); W_res = np.asarray(W_res); primes = np.asarray(primes)
    init_res = np.asarray(init_res)
    readout_W = np.asarray(readout_W, np.float32)
    readout_b = np.asarray(readout_b, np.float32)

    Wp = W_res.astype(np.int64) * primes[None, :].astype(np.int64)
    Wp_keep = Wp.copy(); Wp_keep[:, input_nodes] = 0
    Wp_x = Wp[:, input_nodes]

    def bf16_split(Wint):
        return ((Wint >> 3) << 3).astype(np.float32), (Wint & 7).astype(np.float32)

    A_keep, B_keep = bf16_split(Wp_keep)
    A_x, B_x = bf16_split(Wp_x)

    st_parts = []
    for jh in range(2):
        for kh in range(2):
            for Wm in (A_keep, B_keep):
                st_parts.append(np.ascontiguousarray(
                    Wm[jh*128:(jh+1)*128, kh*128:(kh+1)*128].T).astype(bf))
    st_big = np.concatenate(st_parts, axis=1)                      # [128, 8*128]
    stx_parts = []
    for jh in range(2):
        for Wm in (A_x, B_x):
            stx_parts.append(np.ascontiguousarray(
                Wm[jh*128:(jh+1)*128, :].T).astype(bf))            # [32, 128]
    stx_big = np.tile(np.concatenate(stx_parts, axis=1), (2, 1))   # [64, 4*128]

    lb = lut.astype(np.uint32).reshape(R, NW, 16)
    packed = (lb << np.arange(16, dtype=np.uint32)[None, None, :]).sum(
        axis=2, dtype=np.uint32)
    table = np.empty((128, NE), np.uint32)
    table[:, :NW] = packed[:128]
    table[:, NW:] = packed[128:]

    p_ = np.arange(128)[:, None]; l_ = np.arange(2048)[None, :]
    maskfull = np.where((l_ % 16) == (p_ % 16), np.int32(-1), np.int32(0))

    r0 = np.empty((128, 128), np.float32)
    r0[:, 0::2] = init_res[:128, None].astype(np.float32)
    r0[:, 1::2] = init_res[128:, None].astype(np.float32)
    r0 = r0.astype(bf)

    xf = x.reshape(M, S, KB).astype(np.float32)
    x_cores = []
    for c in range(N_CORES):
        xc = xf[c*PER_CORE:(c+1)*PER_CORE]
        xa = np.zeros((64, (S + 1)//2 * 64), np.float32)
        for t in range(S):
            blk = 32 * (t % 2); col = (t // 2) * 64
            xa[blk:blk+32, col:col+64] = xc[:, t, :].T
        x_cores.append(xa.astype(bf))

    ro = np.zeros((128, 2 * 16), np.float32)
    ro[:, :OUT] = readout_W[:, :128].T
    ro[:, 16:16+OUT] = readout_W[:, 128:].T
    rb = np.zeros((16, 1), np.float32); rb[:OUT, 0] = readout_b

    shared = dict(table=table, st_big=st_big, stx_big=stx_big,
                  maskfull=maskfull, r0=r0, ro=ro, rb=rb)
    shared = {k: np.ascontiguousarray(v) for k, v in shared.items()}
    return shared, x_cores


def _build_kernel(steps=S, name="boolres2"):
    """2-wave pipelined kernel: samples split 2x32; independent state tiles
    per wave so PE/DVE/ACT work of one wave overlaps the other's ap_gather."""
    import concourse.bacc as bacc
    import concourse.tile as tile
    import concourse.mybir as mybir
    from concourse.library_config import ap_gather as apg_lib
    from concourse.alu_op_type import AluOpType

    nc = bacc.Bacc(name=name)
    dt = mybir.dt
    xcols = (steps + 1)//2 * 64
    t_table = nc.dram_tensor("table", [128, NE], dt.uint32, kind="ExternalInput")
    t_st = nc.dram_tensor("st_big", [128, 8*128], dt.bfloat16, kind="ExternalInput")
    t_stx = nc.dram_tensor("stx_big", [64, 4*128], dt.bfloat16, kind="ExternalInput")
    t_mask = nc.dram_tensor("maskfull", [128, 2048], dt.int32, kind="ExternalInput")
    t_r0 = nc.dram_tensor("r0", [128, 128], dt.bfloat16, kind="ExternalInput")
    t_x = nc.dram_tensor("xa", [64, xcols], dt.bfloat16, kind="ExternalInput")
    t_ro = nc.dram_tensor("ro", [128, 32], dt.float32, kind="ExternalInput")
    t_rb = nc.dram_tensor("rb", [16, 1], dt.float32, kind="ExternalInput")
    t_out = nc.dram_tensor("y", [16, 64], dt.float32, kind="ExternalOutput")

    with tile.TileContext(nc) as tc:
        nc.gpsimd.load_library(apg_lib)
        with tc.tile_pool(name="big", bufs=1) as bigp, \
             tc.tile_pool(name="work", bufs=1) as wp, \
             tc.tile_pool(name="ps", bufs=1, space="PSUM") as psp:
            table = bigp.tile([128, NE], dt.uint32, tag="table")
            st = bigp.tile([128, 8*128], dt.bfloat16, tag="st")
            stx = bigp.tile([64, 4*128], dt.bfloat16, tag="stx")
            mask = bigp.tile([128, 2048], dt.int32, tag="mask")
            xa = bigp.tile([64, xcols], dt.bfloat16, tag="xa")
            rclw = [bigp.tile([128, 64], dt.bfloat16, tag=f"rcl{w}", name=f"rcl{w}") for w in range(2)]
            ro = bigp.tile([128, 32], dt.float32, tag="ro")
            rb = bigp.tile([16, 1], dt.float32, tag="rb")
            nc.sync.dma_start(table[:], t_table.ap()[:])
            nc.sync.dma_start(st[:], t_st.ap()[:])
            nc.sync.dma_start(stx[:], t_stx.ap()[:])
            nc.sync.dma_start(mask[:], t_mask.ap()[:])
            nc.sync.dma_start(xa[:], t_x.ap()[:])
            # r0 wave slices: r0 cols [2*32w : 2*32w+64] -> rclw[w]
            nc.sync.dma_start(rclw[0][:], t_r0.ap()[:, 0:64])
            nc.sync.dma_start(rclw[1][:], t_r0.ap()[:, 64:128])
            nc.sync.dma_start(ro[:], t_ro.ap()[:])
            nc.sync.dma_start(rb[:], t_rb.ap()[:])

            def st_slice(jh, kh, nm):
                i = ((jh * 2 + kh) * 2 + nm) * 128
                return st[:, i:i+128]

            def stx_slice(jh, nm, blk):
                i = (jh * 2 + nm) * 128
                return stx[blk:blk+32, i:i+128]

            for t in range(steps):
                for w in range(2):
                    rcl = rclw[w]
                    psl = psp.tile([128, 32], dt.float32, tag=f"psl{w}")
                    psh = psp.tile([128, 32], dt.float32, tag=f"psh{w}")
                    for jh, ps in ((0, psl), (1, psh)):
                        first = True
                        for kh in range(2):
                            mv = rcl[:, kh::2]
                            for nm in range(2):
                                nc.tensor.matmul(out=ps[:], lhsT=st_slice(jh, kh, nm),
                                                 rhs=mv, start=first, stop=False)
                                first = False
                        blk = 32 * (t % 2); col = (t // 2) * 64 + 32 * w
                        xmv = xa[blk:blk+32, col:col+32]
                        nc.tensor.matmul(out=ps[:], lhsT=stx_slice(jh, 0, blk),
                                         rhs=xmv, start=False, stop=False)
                        nc.tensor.matmul(out=ps[:], lhsT=stx_slice(jh, 1, blk),
                                         rhs=xmv, start=False, stop=True)

                    ilo = wp.tile([128, 32], dt.int32, tag=f"ilo{w}")
                    ihi = wp.tile([128, 32], dt.int32, tag=f"ihi{w}")
                    nc.scalar.activation(ilo[:], psl[:], mybir.ActivationFunctionType.Copy)
                    nc.scalar.activation(ihi[:], psh[:], mybir.ActivationFunctionType.Copy)

                    widx32 = wp.tile([128, 64], dt.int32, tag=f"widx32{w}")
                    widx = wp.tile([128, 64], dt.int16, tag=f"widx{w}")
                    b32 = wp.tile([128, 64], dt.int32, tag=f"b32{w}")
                    nc.vector.tensor_scalar(widx32[:, 0::2], ilo[:], 4, None,
                                            AluOpType.logical_shift_right)
                    nc.vector.tensor_scalar(widx32[:, 1::2], ihi[:], 4, None,
                                            AluOpType.logical_shift_right)
                    nc.vector.tensor_scalar(widx32[:, 1::2], widx32[:, 1::2], NW, None,
                                            AluOpType.add)
                    nc.vector.tensor_copy(widx[:], widx32[:])
                    nc.vector.tensor_scalar(b32[:, 0::2], ilo[:], 15, None,
                                            AluOpType.bitwise_and)
                    nc.vector.tensor_scalar(b32[:, 1::2], ihi[:], 15, None,
                                            AluOpType.bitwise_and)
                    eint = wp.tile([128, 64], dt.int32, tag=f"eint{w}")
                    nc.scalar.activation(eint[:], b32[:], mybir.ActivationFunctionType.Copy,
                                         bias=1065353216.0, scale=8388608.0)
                    p2i = wp.tile([128, 64], dt.int32, tag=f"p2i{w}")
                    nc.scalar.activation(p2i[:], eint[:].bitcast(dt.float32),
                                         mybir.ActivationFunctionType.Copy)

                    gout = wp.tile([128, 1024], dt.uint32, tag=f"gout{w}")
                    nc.gpsimd.ap_gather(gout[:], table[:], widx[:],
                                        channels=128, num_elems=NE, d=1, num_idxs=1024)

                    m1 = wp.tile([128, 1024], dt.int32, tag=f"m1{w}")
                    nc.vector.tensor_tensor(m1[:], gout[:].bitcast(dt.int32),
                                            mask[:, 0:1024], op=AluOpType.bitwise_and)
                    m2 = wp.tile([128, 1024], dt.int32, tag=f"m2{w}")
                    p2b = p2i[:].unsqueeze(-1).broadcast_to([128, 64, 16])
                    nc.vector.tensor_tensor(m2[:].rearrange("p (s q) -> p s q", q=16),
                                            m1[:].rearrange("p (s q) -> p s q", q=16),
                                            p2b, op=AluOpType.bitwise_and)
                    rsum = wp.tile([128, 64], dt.float32, tag=f"rsum{w}")
                    nc.vector.tensor_reduce(rsum[:].unsqueeze(-1),
                                            m2[:].rearrange("p (s q) -> p s q", q=16),
                                            axis=mybir.AxisListType.X, op=AluOpType.add)
                    nc.vector.tensor_scalar(rcl[:], rsum[:], 0.0, None,
                                            AluOpType.not_equal)

            psy = psp.tile([16, 64], dt.float32, tag="psy")
            for w in range(2):
                rf32 = wp.tile([128, 64], dt.float32, tag=f"rf32{w}")
                nc.scalar.activation(rf32[:], rclw[w][:], mybir.ActivationFunctionType.Copy)
                nc.tensor.matmul(out=psy[:, 32*w:32*w+32], lhsT=ro[:, 0:16],
                                 rhs=rf32[:, 0::2], start=True, stop=False)
                nc.tensor.matmul(out=psy[:, 32*w:32*w+32], lhsT=ro[:, 16:32],
                                 rhs=rf32[:, 1::2], start=False, stop=True)
            yt = wp.tile([16, 64], dt.float32, tag="yt")
            nc.vector.tensor_scalar(yt[:], psy[:], rb[:], None, AluOpType.add)
            nc.sync.dma_start(t_out.ap()[:], yt[:])
    nc.compile()
    return nc


_CACHE = {}


def kernel(x, input_nodes, lut, W_res, primes, init_res, readout_W, readout_b):
    from concourse.bass_utils import run_bass_kernel_spmd

    shared, x_cores = _host_prep(x, input_nodes, lut, W_res, primes, init_res,
                                 readout_W, readout_b)
    if "nc" not in _CACHE:
        _CACHE["nc"] = _build_kernel(steps=S, name="boolres512")
    nc = _CACHE["nc"]
    in_maps = [{**shared, "xa": x_cores[c]} for c in range(N_CORES)]
    res = run_bass_kernel_spmd(nc, in_maps, core_ids=list(range(N_CORES)))
    ys = [res.results[c]["y"] for c in range(N_CORES)]       # each [16, 64]
    out = np.concatenate([y[:OUT, :].T for y in ys], axis=0)  # [512, 10]
    return np.ascontiguousarray(out.astype(np.float32))



# revision 11
# speedup vs baseline: 1.7274x; 1.7274x over previous
"""Boolean reservoir computing on 8 Trainium2 NeuronCores (Bass/Tile).

Algorithm per step (data-parallel over samples, 64 per core):
  - exact integer indices idx = r @ Wp.T via bf16-split matmuls accumulated
    in fp32 PSUM (input-bit injection folded into separate stationary
    weights, so no state scatter is needed)
  - LUT lookup from an SBUF-resident bitpacked table (16 payload bits per
    uint32 word, one node's table per partition) via the GPSIMD ap_gather
    instruction; per-lookup bit extracted with integer ALU ops and a
    power-of-two AND mask built from exponent-field arithmetic
  - final readout matmul in fp32 on-device
"""
import numpy as np

R = 256; LUT_LEN = 18; M = 512; S = 512; OUT = 10
KB = 32
NW = 1 << (LUT_LEN - 4)     # 16384 16-bit words per node
NE = 2 * NW                 # uint32 entries per partition (2 nodes)
N_CORES = 8
PER_CORE = M // N_CORES     # 64


def _host_prep(x, input_nodes, lut, W_res, primes, init_res, readout_W, readout_b):
    import ml_dtypes
    bf = ml_dtypes.bfloat16
    x = np.asarray(x); input_nodes = np.asarray(input_nodes)
    lut = np.asarray(lut); W_res = np.asarray(W_res); primes = np.asarray(primes)
    init_res = np.asarray(init_res)
    readout_W = np.asarray(readout_W, np.float32)
    readout_b = np.asarray(readout_b, np.float32)

    Wp = W_res.astype(np.int64) * primes[None, :].astype(np.int64)
    Wp_keep = Wp.copy(); Wp_keep[:, input_nodes] = 0
    Wp_x = Wp[:, input_nodes]

    def bf16_split(Wint):
        return ((Wint >> 3) << 3).astype(np.float32), (Wint & 7).astype(np.float32)

    A_keep, B_keep = bf16_split(Wp_keep)
    A_x, B_x = bf16_split(Wp_x)

    st_parts = []
    for jh in range(2):
        for kh in range(2):
            for Wm in (A_keep, B_keep):
                st_parts.append(np.ascontiguousarray(
                    Wm[jh*128:(jh+1)*128, kh*128:(kh+1)*128].T).astype(bf))
    st_big = np.concatenate(st_parts, axis=1)                      # [128, 8*128]
    stx_parts = []
    for jh in range(2):
        for Wm in (A_x, B_x):
            stx_parts.append(np.ascontiguousarray(
                Wm[jh*128:(jh+1)*128, :].T).astype(bf))            # [32, 128]
    stx_big = np.tile(np.concatenate(stx_parts, axis=1), (2, 1))   # [64, 4*128]

    lb = lut.astype(np.uint32).reshape(R, NW, 16)
    packed = (lb << np.arange(16, dtype=np.uint32)[None, None, :]).sum(
        axis=2, dtype=np.uint32)
    table = np.empty((128, NE), np.uint32)
    table[:, :NW] = packed[:128]
    table[:, NW:] = packed[128:]

    p_ = np.arange(128)[:, None]; l_ = np.arange(2048)[None, :]
    maskfull = np.where((l_ % 16) == (p_ % 16), np.int32(-1), np.int32(0))

    r0 = np.empty((128, 128), np.float32)
    r0[:, 0::2] = init_res[:128, None].astype(np.float32)
    r0[:, 1::2] = init_res[128:, None].astype(np.float32)
    r0 = r0.astype(bf)

    xf = x.reshape(M, S, KB).astype(np.float32)
    x_cores = []
    for c in range(N_CORES):
        xc = xf[c*PER_CORE:(c+1)*PER_CORE]
        xa = np.zeros((64, (S + 1)//2 * 64), np.float32)
        for t in range(S):
            blk = 32 * (t % 2); col = (t // 2) * 64
            xa[blk:blk+32, col:col+64] = xc[:, t, :].T
        x_cores.append(xa.astype(bf))

    ro = np.zeros((128, 2 * 16), np.float32)
    ro[:, :OUT] = readout_W[:, :128].T
    ro[:, 16:16+OUT] = readout_W[:, 128:].T
    rb = np.zeros((16, 1), np.float32); rb[:OUT, 0] = readout_b

    shared = dict(table=table, st_big=st_big, stx_big=stx_big,
                  maskfull=maskfull, r0=r0, ro=ro, rb=rb)
    shared = {k: np.ascontiguousarray(v) for k, v in shared.items()}
    return shared, x_cores


def _build_kernel(steps=S, name="boolres2", waves=4):
    """Pipelined kernel: samples split into `waves` groups with independent
    state tiles, so PE/DVE/ACT work of one wave overlaps the others'
    ap_gather. 4 waves of 512-idx gathers measure ~10% cheaper per index
    than 2 waves of 1024 and pipeline tighter."""
    import concourse.bacc as bacc
    import concourse.tile as tile
    import concourse.mybir as mybir
    from concourse.library_config import ap_gather as apg_lib
    from concourse.alu_op_type import AluOpType

    nc = bacc.Bacc(name=name)
    dt = mybir.dt
    W = waves
    SPW = 64 // W                 # samples per wave
    NI = 32 * SPW                 # gather idx slots per wave (2*SPW cols x 16)
    xcols = (steps + 1)//2 * 64
    t_table = nc.dram_tensor("table", [128, NE], dt.uint32, kind="ExternalInput")
    t_st = nc.dram_tensor("st_big", [128, 8*128], dt.bfloat16, kind="ExternalInput")
    t_stx = nc.dram_tensor("stx_big", [64, 4*128], dt.bfloat16, kind="ExternalInput")
    t_mask = nc.dram_tensor("maskfull", [128, 2048], dt.int32, kind="ExternalInput")
    t_r0 = nc.dram_tensor("r0", [128, 128], dt.bfloat16, kind="ExternalInput")
    t_x = nc.dram_tensor("xa", [64, xcols], dt.bfloat16, kind="ExternalInput")
    t_ro = nc.dram_tensor("ro", [128, 32], dt.float32, kind="ExternalInput")
    t_rb = nc.dram_tensor("rb", [16, 1], dt.float32, kind="ExternalInput")
    t_out = nc.dram_tensor("y", [16, 64], dt.float32, kind="ExternalOutput")

    with tile.TileContext(nc) as tc:
        nc.gpsimd.load_library(apg_lib)
        with tc.tile_pool(name="big", bufs=1) as bigp, \
             tc.tile_pool(name="work", bufs=1) as wp, \
             tc.tile_pool(name="ps", bufs=1, space="PSUM") as psp:
            table = bigp.tile([128, NE], dt.uint32, tag="table")
            st = bigp.tile([128, 8*128], dt.bfloat16, tag="st")
            stx = bigp.tile([64, 4*128], dt.bfloat16, tag="stx")
            mask = bigp.tile([128, 2048], dt.int32, tag="mask")
            xa = bigp.tile([64, xcols], dt.bfloat16, tag="xa")
            rclw = [bigp.tile([128, 2 * SPW], dt.bfloat16, tag=f"rcl{w}", name=f"rcl{w}") for w in range(W)]
            ro = bigp.tile([128, 32], dt.float32, tag="ro")
            rb = bigp.tile([16, 1], dt.float32, tag="rb")
            nc.sync.dma_start(table[:], t_table.ap()[:])
            nc.sync.dma_start(st[:], t_st.ap()[:])
            nc.sync.dma_start(stx[:], t_stx.ap()[:])
            nc.sync.dma_start(mask[:], t_mask.ap()[:])
            nc.sync.dma_start(xa[:], t_x.ap()[:])
            # r0 wave slices: r0 cols [2*SPW*w : 2*SPW*(w+1)] -> rclw[w]
            for w in range(W):
                nc.sync.dma_start(rclw[w][:],
                                  t_r0.ap()[:, 2*SPW*w:2*SPW*(w+1)])
            nc.sync.dma_start(ro[:], t_ro.ap()[:])
            nc.sync.dma_start(rb[:], t_rb.ap()[:])

            def st_slice(jh, kh, nm):
                i = ((jh * 2 + kh) * 2 + nm) * 128
                return st[:, i:i+128]

            def stx_slice(jh, nm, blk):
                i = (jh * 2 + nm) * 128
                return stx[blk:blk+32, i:i+128]

            for t in range(steps):
                for w in range(W):
                    rcl = rclw[w]
                    pslh = psp.tile([128, 2*SPW], dt.float32, tag=f"pslh{w}")
                    psl = pslh[:, 0:SPW]
                    psh = pslh[:, SPW:2*SPW]
                    for jh, ps in ((0, psl), (1, psh)):
                        first = True
                        for kh in range(2):
                            mv = rcl[:, kh::2]
                            for nm in range(2):
                                nc.tensor.matmul(out=ps, lhsT=st_slice(jh, kh, nm),
                                                 rhs=mv, start=first, stop=False)
                                first = False
                        blk = 32 * (t % 2); col = (t // 2) * 64 + SPW * w
                        xmv = xa[blk:blk+32, col:col+SPW]
                        nc.tensor.matmul(out=ps, lhsT=stx_slice(jh, 0, blk),
                                         rhs=xmv, start=False, stop=False)
                        nc.tensor.matmul(out=ps, lhsT=stx_slice(jh, 1, blk),
                                         rhs=xmv, start=False, stop=True)

                    ilo = wp.tile([128, SPW], dt.int32, tag=f"ilo{w}")
                    ihi = wp.tile([128, SPW], dt.int32, tag=f"ihi{w}")
                    nc.scalar.activation(ilo[:], psl, mybir.ActivationFunctionType.Copy)
                    nc.scalar.activation(ihi[:], psh, mybir.ActivationFunctionType.Copy)

                    widx32 = wp.tile([128, 2*SPW], dt.int32, tag=f"widx32{w}")
                    widx = wp.tile([128, 2*SPW], dt.int16, tag=f"widx{w}")
                    b32 = wp.tile([128, 2*SPW], dt.int32, tag=f"b32{w}")
                    nc.vector.tensor_scalar(widx32[:, 0::2], ilo[:], 4, None,
                                            AluOpType.logical_shift_right)
                    nc.vector.tensor_scalar(widx32[:, 1::2], ihi[:], 4, None,
                                            AluOpType.logical_shift_right)
                    nc.vector.tensor_scalar(widx32[:, 1::2], widx32[:, 1::2], NW, None,
                                            AluOpType.add)
                    nc.vector.tensor_copy(widx[:], widx32[:])
                    nc.vector.tensor_scalar(b32[:, 0::2], ilo[:], 15, None,
                                            AluOpType.bitwise_and)
                    nc.vector.tensor_scalar(b32[:, 1::2], ihi[:], 15, None,
                                            AluOpType.bitwise_and)
                    eint = wp.tile([128, 2*SPW], dt.int32, tag=f"eint{w}")
                    nc.scalar.activation(eint[:], b32[:], mybir.ActivationFunctionType.Copy,
                                         bias=1065353216.0, scale=8388608.0)
                    p2i = wp.tile([128, 2*SPW], dt.int32, tag=f"p2i{w}")
                    nc.scalar.activation(p2i[:], eint[:].bitcast(dt.float32),
                                         mybir.ActivationFunctionType.Copy)

                    gout = wp.tile([128, NI], dt.uint32, tag=f"gout{w}")
                    nc.gpsimd.ap_gather(gout[:], table[:], widx[:],
                                        channels=128, num_elems=NE, d=1, num_idxs=NI)

                    m1 = wp.tile([128, NI], dt.int32, tag=f"m1{w}")
                    nc.vector.tensor_tensor(m1[:], gout[:].bitcast(dt.int32),
                                            mask[:, 0:NI], op=AluOpType.bitwise_and)
                    m2 = wp.tile([128, NI], dt.int32, tag=f"m2{w}")
                    p2b = p2i[:].unsqueeze(-1).broadcast_to([128, 2*SPW, 16])
                    nc.vector.tensor_tensor(m2[:].rearrange("p (s q) -> p s q", q=16),
                                            m1[:].rearrange("p (s q) -> p s q", q=16),
                                            p2b, op=AluOpType.bitwise_and)
                    rsum = wp.tile([128, 2*SPW], dt.float32, tag=f"rsum{w}")
                    nc.vector.tensor_reduce(rsum[:].unsqueeze(-1),
                                            m2[:].rearrange("p (s q) -> p s q", q=16),
                                            axis=mybir.AxisListType.X, op=AluOpType.add)
                    nc.vector.tensor_scalar(rcl[:], rsum[:], 0.0, None,
                                            AluOpType.not_equal)

            psy = psp.tile([16, 64], dt.float32, tag="psy")
            for w in range(W):
                rf32 = wp.tile([128, 2*SPW], dt.float32, tag=f"rf32{w}")
                nc.scalar.activation(rf32[:], rclw[w][:], mybir.ActivationFunctionType.Copy)
                nc.tensor.matmul(out=psy[:, SPW*w:SPW*w+SPW], lhsT=ro[:, 0:16],
                                 rhs=rf32[:, 0::2], start=True, stop=False)
                nc.tensor.matmul(out=psy[:, SPW*w:SPW*w+SPW], lhsT=ro[:, 16:32],
                                 rhs=rf32[:, 1::2], start=False, stop=True)
            yt = wp.tile([16, 64], dt.float32, tag="yt")
            nc.vector.tensor_scalar(yt[:], psy[:], rb[:], None, AluOpType.add)
            nc.sync.dma_start(t_out.ap()[:], yt[:])
    nc.compile()
    return nc


_CACHE = {}


def kernel(x, input_nodes, lut, W_res, primes, init_res, readout_W, readout_b):
    from concourse.bass_utils import run_bass_kernel_spmd

    shared, x_cores = _host_prep(x, input_nodes, lut, W_res, primes, init_res,
                                 readout_W, readout_b)
    if "nc" not in _CACHE:
        _CACHE["nc"] = _build_kernel(steps=S, name="boolres512")
    nc = _CACHE["nc"]
    in_maps = [{**shared, "xa": x_cores[c]} for c in range(N_CORES)]
    res = run_bass_kernel_spmd(nc, in_maps, core_ids=list(range(N_CORES)))
    ys = [res.results[c]["y"] for c in range(N_CORES)]       # each [16, 64]
    out = np.concatenate([y[:OUT, :].T for y in ys], axis=0)  # [512, 10]
    return np.ascontiguousarray(out.astype(np.float32))

